# revision 1
# baseline (speedup 1.0000x reference)
"""Trainium2 Bass kernel for nn_LlamaMLP (BitLinear-style ternary-quantized MLP).

Reference computation (all f32):
    s_m   = mean(|w_m|)                            (global scalar per weight)
    q_m   = round(clip(w_m / (s_m + eps), -1, 1))  (ternary)
    gate  = x @ (q_g * s_g).T ; up = x @ (q_u * s_u).T
    out   = (gate * up) @ (q_d * s_d).T
        == (s_g*s_u*s_d) * ((x @ q_g.T) * (x @ q_u.T)) @ q_d.T

Strategy: tensor-parallel over the intermediate dim I (padded to a multiple of
128*n_cores). Each core receives transposed f32 weight shards, computes a
partial |w| sum (3 scalars, AllReduce'd for the global means), quantizes its
shards to exactly-representable ternary bf16 on device, runs the three matmuls
in bf16 with f32 PSUM accumulation, and the partial [T, H] output is
ReduceScatter'd per 512-token block (pipelined behind the compute).

The host wrapper does layout work (transpose / zero-pad / slice / concat) plus
the f32->bf16 cast of the activations (bit-identical to doing the cast on
device; weights stay f32 so quantization matches the reference).
"""

import sys

sys.path.insert(0, "/opt/trn_rl_repo")

import numpy as np
import concourse.mybir as mybir
import concourse.tile as tile
import concourse.bass_isa as bass_isa
from concourse import bacc
from concourse.bass_utils import run_bass_kernel_spmd

F32 = mybir.dt.float32
BF16 = mybir.dt.bfloat16
ALU = mybir.AluOpType
AX = mybir.AxisListType
ACTF = mybir.ActivationFunctionType

P = 128
TB = 512  # token-block width (matmul moving free dim)
MAGIC = 12582912.0  # 1.5*2^23; add+sub rounds an f32 to nearest-even integer
EPS = 1e-5

# Full-problem config
FULL_T, FULL_H, FULL_I = 8192, 4096, 11008
N_CORES = 8

# Filled by kernel(); read by test.py
LAST_RESULTS = None


def shard_sizes(I_real, n_cores):
    i_s = -(-I_real // (P * n_cores)) * P  # per-core padded shard (mult of 128)
    return i_s, i_s // P


def build_bass(T=FULL_T, H=FULL_H, I_real=FULL_I, n_cores=N_CORES):
    assert T % TB == 0 and H % P == 0 and H % TB == 0 and TB % n_cores == 0
    HT = H // P  # contraction tiles for gate/up
    HB = H // TB  # down-phase output column blocks
    NB = T // TB  # token blocks
    TS = TB // P  # token sub-tiles per block (down-phase lhsT)
    i_s, IT = shard_sizes(I_real, n_cores)
    nreal = I_real * H  # real element count of each weight matrix
    rq = TB // n_cores  # ReduceScatter rows per core per block

    nc = bacc.Bacc("TRN2", target_bir_lowering=False, debug=False, num_devices=n_cores)
    xTb = nc.dram_tensor("xTb", [H, T], BF16, kind="ExternalInput")
    wgT = nc.dram_tensor("wgT", [H, i_s], F32, kind="ExternalInput")
    wuT = nc.dram_tensor("wuT", [H, i_s], F32, kind="ExternalInput")
    wdT = nc.dram_tensor("wdT", [i_s, H], F32, kind="ExternalInput")
    y = nc.dram_tensor("y", [NB, rq, H], F32, kind="ExternalOutput")
    rg = [list(range(n_cores))]

    with tile.TileContext(nc) as tc:
        with tc.tile_pool(name="dram", bufs=1, space="DRAM") as dram:
            # quantized weights, i-major: column i of lhsT tiles is contiguous
            # per partition for the streaming reads in phase C
            qg_d = dram.tile([IT, P, HT * P], BF16)
            qu_d = dram.tile([IT, P, HT * P], BF16)
            qd_d = dram.tile([IT, P, H], BF16)  # down rhs tiles
            # per-block output buffers: separate tiles so block b's ReduceScatter
            # doesn't serialize against block b+1's output DMAs (whole-tile deps)
            outb = [
                dram.tile([TB, H], F32, name=f"outb{b}", tag=f"outb{b}")
                for b in range(NB)
            ]
            rsb = [
                dram.tile([rq, H], F32, name=f"rsb{b}", tag=f"rsb{b}") for b in range(NB)
            ]
            cc_in = dram.tile([1, 8], F32)
            cc_out = dram.tile([1, 8], F32, addr_space="Shared")

            with tc.tile_pool(name="res", bufs=1) as rpool:
                rdenb = rpool.tile([P, 4], F32)  # 1/(s_m + eps), broadcast
                cb = rpool.tile([P, 1], F32)  # s_g*s_u*s_d, broadcast
                acc = rpool.tile([P, 4], F32)  # per-partition |w| sums
                sums = rpool.tile([1, 8], F32)
                gsums = rpool.tile([1, 8], F32)
                den = rpool.tile([1, 4], F32)
                rden = rpool.tile([1, 4], F32)
                s3 = rpool.tile([1, 4], F32)
                cprod = rpool.tile([1, 1], F32)

                srcs = [(wgT, HT, i_s), (wuT, HT, i_s), (wdT, IT, H)]

                # ---------- Phase A: global scales ----------
                with tc.tile_pool(name="scale", bufs=4) as spool:
                    nc.vector.memset(acc, 0.0)
                    for m, (w, rows, cols) in enumerate(srcs):
                        for r in range(0, rows, 2):  # up to 2 row-tiles per DMA
                            g = min(2, rows - r)
                            st = spool.tile(
                                [P, 2, cols], F32, tag="sst", name=f"sst{m}_{r}"
                            )
                            nc.sync.dma_start(
                                st[:, :g, :],
                                w[r * P : (r + g) * P, :].rearrange(
                                    "(g p) c -> p g c", p=P
                                ),
                            )
                            part = spool.tile([P, 1], F32, tag="sp", name=f"sp{m}_{r}")
                            nc.vector.tensor_reduce(
                                part,
                                st[:, :g, :],
                                axis=AX.XY,
                                op=ALU.add,
                                apply_absolute_value=True,
                            )
                            nc.vector.tensor_tensor(
                                acc[:, m : m + 1], acc[:, m : m + 1], part, op=ALU.add
                            )
                    nc.vector.memset(sums, 0.0)
                    for m in range(3):
                        allb = spool.tile([P, 1], F32, tag="allb", name=f"allb{m}")
                        nc.gpsimd.partition_all_reduce(
                            allb, acc[:, m : m + 1], P, bass_isa.ReduceOp.add
                        )
                        nc.vector.tensor_copy(sums[0:1, m : m + 1], allb[0:1, 0:1])
                    nc.sync.dma_start(cc_in[:], sums[:])
                    nc.gpsimd.collective_compute(
                        "AllReduce",
                        ALU.add,
                        ins=[cc_in[:]],
                        outs=[cc_out[:]],
                        replica_groups=rg,
                    )
                    nc.sync.dma_start(gsums[:], cc_out[:])
                    rn = 1.0 / float(nreal)
                    nc.vector.tensor_scalar(
                        den[0:1, 0:3], gsums[0:1, 0:3], rn, EPS, ALU.mult, ALU.add
                    )
                    nc.vector.reciprocal(rden[0:1, 0:3], den[0:1, 0:3])
                    nc.vector.tensor_scalar(
                        s3[0:1, 0:3], gsums[0:1, 0:3], rn, None, ALU.mult
                    )
                    nc.vector.tensor_tensor(cprod, s3[0:1, 0:1], s3[0:1, 1:2], op=ALU.mult)
                    nc.vector.tensor_tensor(cprod, cprod, s3[0:1, 2:3], op=ALU.mult)
                    nc.gpsimd.partition_broadcast(rdenb, rden)
                    nc.gpsimd.partition_broadcast(cb, cprod)

                # ---------- Phase B: quantize shards to ternary bf16 ----------
                # ACT does w*r+MAGIC (f32 add rounds to nearest-even integer),
                # DVE does -MAGIC & clamp low, then clamp high + bf16 cast.
                def qround(dst, src, m, pool, cols, nm):
                    t1 = pool.tile([P, cols], F32, tag=f"qt{cols}", name=f"qt_{nm}")
                    nc.scalar.activation(
                        t1, src, ACTF.Copy, bias=MAGIC, scale=rdenb[:, m : m + 1]
                    )
                    nc.vector.tensor_scalar(t1, t1, MAGIC, -1.0, ALU.subtract, ALU.max)
                    nc.vector.tensor_scalar(dst, t1, 1.0, None, ALU.min)

                with tc.tile_pool(name="quant", bufs=3) as qpool:
                    for m, (w, qdst) in enumerate([(wgT, qg_d), (wuT, qu_d)]):
                        for h in range(HT):
                            st = qpool.tile([P, i_s], F32, tag="qsg", name=f"qs{m}_{h}")
                            nc.sync.dma_start(st[:], w[h * P : (h + 1) * P, :])
                            qb = qpool.tile([P, i_s], BF16, tag="qbu", name=f"qb{m}_{h}")
                            qround(qb, st, m, qpool, i_s, f"{m}_{h}")
                            nc.sync.dma_start(
                                qdst[:, :, h * P : (h + 1) * P].rearrange(
                                    "i p f -> p i f"
                                ),
                                qb.rearrange("p (i f) -> p i f", i=IT),
                            )
                    CH = min(H, 2048)
                    for it in range(IT):  # down -> [IT, P, H]
                        for c0 in range(0, H, CH):
                            st = qpool.tile([P, CH], F32, tag="qsd", name=f"qsd{it}_{c0}")
                            nc.sync.dma_start(
                                st[:], wdT[it * P : (it + 1) * P, c0 : c0 + CH]
                            )
                            qb = qpool.tile([P, CH], BF16, tag="qbd", name=f"qbd{it}_{c0}")
                            qround(qb, st, 2, qpool, CH, f"d{it}_{c0}")
                            nc.sync.dma_start(qd_d[it, :, c0 : c0 + CH], qb[:])

                # ---------- Phase C: main loop over token blocks ----------
                with (
                    tc.tile_pool(name="main", bufs=2) as mpool,
                    tc.tile_pool(name="ps", bufs=8, space="PSUM") as pspool,
                ):
                    for b in range(NB):
                        xb = mpool.tile([P, HT, TB], BF16, tag="xb", bufs=2, name=f"xb{b}")
                        nc.sync.dma_start(
                            xb[:],
                            xTb[:, b * TB : (b + 1) * TB].rearrange(
                                "(g p) f -> p g f", p=P
                            ),
                        )
                        interT = mpool.tile(
                            [P, IT, TB], BF16, tag="inter", bufs=1, name=f"int{b}"
                        )
                        for i in range(IT):
                            qgc = mpool.tile(
                                [P, HT * P], BF16, tag="qgc", bufs=2, name=f"qgc{b}_{i}"
                            )
                            nc.sync.dma_start(qgc[:], qg_d[i])
                            quc = mpool.tile(
                                [P, HT * P], BF16, tag="quc", bufs=2, name=f"quc{b}_{i}"
                            )
                            nc.sync.dma_start(quc[:], qu_d[i])
                            pg = pspool.tile([P, TB], F32, tag="ps", name=f"pg{b}_{i}")
                            for h in range(HT):
                                nc.tensor.matmul(
                                    pg,
                                    lhsT=qgc[:, h * P : (h + 1) * P],
                                    rhs=xb[:, h, :],
                                    start=(h == 0),
                                    stop=(h == HT - 1),
                                )
                            pu = pspool.tile([P, TB], F32, tag="ps", name=f"pu{b}_{i}")
                            for h in range(HT):
                                nc.tensor.matmul(
                                    pu,
                                    lhsT=quc[:, h * P : (h + 1) * P],
                                    rhs=xb[:, h, :],
                                    start=(h == 0),
                                    stop=(h == HT - 1),
                                )
                            # up PSUM -> SBUF on ACT (keeps DVE to 1 PSUM read)
                            usb = mpool.tile([P, TB], F32, tag="usb", bufs=2, name=f"usb{b}_{i}")
                            nc.scalar.activation(usb, pu, ACTF.Copy)
                            nc.vector.tensor_tensor(
                                interT[:, i, :], pg, usb, op=ALU.mult
                            )
                        for hb in range(HB):
                            qdc = mpool.tile(
                                [P, IT, TB], BF16, tag="qdc", bufs=2, name=f"qdc{b}_{hb}"
                            )
                            nc.sync.dma_start(
                                qdc[:],
                                qd_d[:, :, hb * TB : (hb + 1) * TB].rearrange(
                                    "i p f -> p i f"
                                ),
                            )
                            pos = [
                                pspool.tile([P, TB], F32, tag="ps", name=f"po{b}_{hb}_{t}")
                                for t in range(TS)
                            ]
                            for i in range(IT):
                                for ts in range(TS):
                                    nc.tensor.matmul(
                                        pos[ts],
                                        lhsT=interT[:, i, ts * P : (ts + 1) * P],
                                        rhs=qdc[:, i, :],
                                        start=(i == 0),
                                        stop=(i == IT - 1),
                                    )
                            ob = mpool.tile(
                                [P, TS, TB], F32, tag="ob", bufs=2, name=f"ob{b}_{hb}"
                            )
                            for ts in range(TS):
                                nc.vector.tensor_scalar(
                                    ob[:, ts, :], pos[ts], cb[:, 0:1], None, ALU.mult
                                )
                            nc.sync.dma_start(
                                outb[b][:, hb * TB : (hb + 1) * TB].rearrange(
                                    "(g p) f -> p g f", p=P
                                ),
                                ob[:],
                            )
                        # pipelined ReduceScatter of this block's partial output
                        nc.gpsimd.collective_compute(
                            "ReduceScatter",
                            ALU.add,
                            ins=[outb[b][:]],
                            outs=[rsb[b][:]],
                            replica_groups=rg,
                        )
                        nc.sync.dma_start(y[b], rsb[b][:])
    nc.compile()
    return nc


_NC_CACHE = {}


def _get_nc(T, H, I_real, n_cores):
    key = (T, H, I_real, n_cores)
    if key not in _NC_CACHE:
        _NC_CACHE[key] = build_bass(T, H, I_real, n_cores)
    return _NC_CACHE[key]


def shard_inputs(hidden_states, w_gate, w_up, w_down, n_cores=N_CORES):
    """Host prep: flatten/transpose/zero-pad/slice; activations cast to bf16
    (bit-identical to the on-device cast the kernel would otherwise do)."""
    B, S, H = hidden_states.shape
    T = B * S
    I_real = w_gate.shape[0]
    i_s, _ = shard_sizes(I_real, n_cores)
    Ip = i_s * n_cores
    bf16 = mybir.dt.np(BF16)

    xTb = np.ascontiguousarray(
        hidden_states.reshape(T, H).T.astype(np.float32, copy=False)
    ).astype(bf16)
    wgT = np.zeros((H, Ip), np.float32)
    wgT[:, :I_real] = w_gate.T
    wuT = np.zeros((H, Ip), np.float32)
    wuT[:, :I_real] = w_up.T
    wdT = np.zeros((Ip, H), np.float32)
    wdT[:I_real, :] = w_down.T

    in_maps = []
    for c in range(n_cores):
        in_maps.append(
            {
                "xTb": xTb,
                "wgT": np.ascontiguousarray(wgT[:, c * i_s : (c + 1) * i_s]),
                "wuT": np.ascontiguousarray(wuT[:, c * i_s : (c + 1) * i_s]),
                "wdT": np.ascontiguousarray(wdT[c * i_s : (c + 1) * i_s, :]),
            }
        )
    return in_maps, (B, S, H, T)


def kernel(hidden_states, w_gate, w_up, w_down, _trace=False):
    global LAST_RESULTS
    n_cores = N_CORES
    in_maps, (B, S, H, T) = shard_inputs(hidden_states, w_gate, w_up, w_down, n_cores)
    I_real = w_gate.shape[0]
    nc = _get_nc(T, H, I_real, n_cores)
    res = run_bass_kernel_spmd(
        nc, in_maps, core_ids=list(range(n_cores)), trace=_trace
    )
    LAST_RESULTS = res

    NB = T // TB
    rq = TB // n_cores
    out = np.empty((T, H), np.float32)
    for c in range(n_cores):
        yc = res.results[c]["y"]  # [NB, rq, H]
        for b in range(NB):
            out[b * TB + c * rq : b * TB + (c + 1) * rq] = yc[b]
    return out.reshape(B, S, H)



# revision 3
# speedup vs baseline: 1.0599x; 1.0599x over previous
"""Trainium2 Bass kernel for nn_LlamaMLP (BitLinear-style ternary-quantized MLP).

Reference computation (all f32):
    s_m   = mean(|w_m|)                            (global scalar per weight)
    q_m   = round(clip(w_m / (s_m + eps), -1, 1))  (ternary)
    gate  = x @ (q_g * s_g).T ; up = x @ (q_u * s_u).T
    out   = (gate * up) @ (q_d * s_d).T
        == (s_g*s_u*s_d) * ((x @ q_g.T) * (x @ q_u.T)) @ q_d.T

Strategy: tensor-parallel over the intermediate dim I (padded to a multiple of
128*n_cores).  Per core:

  Phase A   stream all three f32 weight shards once, reduce |w| partial sums
            (DVE), partition-reduce (GPSIMD), one 8-core AllReduce -> global
            scales.  Pair-0 x blocks prefetch concurrently on the scalar
            HWDGE queue.
  Phase B   re-stream the shards i-tile-major and quantize to ternary bf16:
            ACT (w*rden + MAGIC), DVE (sub MAGIC + clamp lo, in place), then
            GPSIMD (clamp hi + bf16 cast) for gate/up or DVE for down.
            Engine split keeps DVE/PE free of B back-pressure.
  Phase C   token-block-pair compute.  Pair 0 consumes the quantized gate/up
            tiles straight out of SBUF, chasing phase B tile-by-tile; later
            pairs re-read them from DRAM (written once by B).  Gate/up
            matmuls accumulate over H into PSUM; inter = pg*pu (DVE) in bf16;
            down matmuls accumulate over I; bf16 partial outputs are
            ReduceScatter'd per 512-token block (pipelined behind compute).

The host wrapper does layout only (transpose / zero-pad / block / concat plus
the f32->bf16 x cast, bit-identical to an on-device cast; weights stay f32 so
on-device quantization matches the reference).
"""

import sys

sys.path.insert(0, "/opt/trn_rl_repo")

import numpy as np
import concourse.mybir as mybir
import concourse.tile as tile
import concourse.bass_isa as bass_isa
from concourse import bacc
from concourse.bass_utils import run_bass_kernel_spmd

F32 = mybir.dt.float32
BF16 = mybir.dt.bfloat16
ALU = mybir.AluOpType
AX = mybir.AxisListType
ACTF = mybir.ActivationFunctionType

P = 128
TB = 512  # token-block width (matmul moving free dim)
MAGIC = 12582912.0  # 1.5*2^23; add+sub rounds an f32 to nearest-even integer
EPS = 1e-5

FULL_T, FULL_H, FULL_I = 8192, 4096, 11008
N_CORES = 8

LAST_RESULTS = None  # read by test.py


def shard_sizes(I_real, n_cores):
    i_s = -(-I_real // (P * n_cores)) * P  # per-core padded shard (mult of 128)
    return i_s, i_s // P


def build_bass(T=FULL_T, H=FULL_H, I_real=FULL_I, n_cores=N_CORES):
    assert T % (2 * TB) == 0 and H % P == 0 and H % TB == 0 and TB % n_cores == 0
    HT = H // P  # contraction tiles for gate/up
    HB = H // TB  # down-phase output column blocks
    NB = T // TB  # token blocks
    NPAIR = NB // 2
    TS = TB // P  # token sub-tiles per block (down-phase lhsT)
    i_s, IT = shard_sizes(I_real, n_cores)
    nreal = I_real * H
    rq = TB // n_cores  # ReduceScatter rows per core per block
    rn = 1.0 / float(nreal)

    nc = bacc.Bacc("TRN2", target_bir_lowering=False, debug=False, num_devices=n_cores)
    # i-tile-major blocked weights: w*[it][p, g*P+c] = w^T[g*P+p, it*P+c]
    xTb = nc.dram_tensor("xTb", [H, T], BF16, kind="ExternalInput")
    wg = nc.dram_tensor("wg", [IT, P, H], F32, kind="ExternalInput")
    wu = nc.dram_tensor("wu", [IT, P, H], F32, kind="ExternalInput")
    wd = nc.dram_tensor("wd", [IT, P, H], F32, kind="ExternalInput")
    y = nc.dram_tensor("y", [NB, rq, H], BF16, kind="ExternalOutput")
    rg = [list(range(n_cores))]

    with tile.TileContext(nc) as tc:
        with tc.tile_pool(name="dram", bufs=1, space="DRAM") as dram:
            qg_d = [dram.tile([P, H], BF16, name=f"qg{i}", tag=f"qg{i}") for i in range(IT)]
            qu_d = [dram.tile([P, H], BF16, name=f"qu{i}", tag=f"qu{i}") for i in range(IT)]
            qd_d = dram.tile([P, IT, H], BF16)
            outb = [
                dram.tile([TB, H], BF16, name=f"outb{b}", tag=f"outb{b}")
                for b in range(NB)
            ]
            rsb = [
                dram.tile([rq, H], BF16, name=f"rsb{b}", tag=f"rsb{b}")
                for b in range(NB)
            ]
            cc_in = dram.tile([1, 8], F32)
            cc_out = dram.tile([1, 8], F32, addr_space="Shared")

            with (
                tc.tile_pool(name="res", bufs=1) as rpool,
                tc.tile_pool(name="wp", bufs=3) as wpool,
                tc.tile_pool(name="qp", bufs=2) as qpool,
                tc.tile_pool(name="mp", bufs=1) as mpool,
                tc.tile_pool(name="ps", bufs=8, space="PSUM") as pspool,
            ):
                rdenb = rpool.tile([P, 4], F32)  # 1/(s_m+eps) bcast (cols g,u,d)
                cb = rpool.tile([P, 1], F32)  # s_g*s_u*s_d bcast
                acc = rpool.tile([P, 4], F32)
                sums = rpool.tile([1, 8], F32)
                gsums = rpool.tile([1, 8], F32)
                den = rpool.tile([1, 4], F32)
                rden = rpool.tile([1, 4], F32)
                s3 = rpool.tile([1, 4], F32)
                cprod = rpool.tile([1, 1], F32)

                # pair-0 x blocks: scalar HWDGE queue, issue at t=0 under A
                xb0 = [mpool.tile([P, HT, TB], BF16, tag=f"xb{k}", bufs=1,
                                  name=f"xb0_{k}") for k in range(2)]
                for k in range(2):
                    nc.scalar.dma_start(
                        xb0[k],
                        xTb[:, k * TB : (k + 1) * TB].rearrange("(g p) f -> p g f", p=P),
                    )

                # ---------- Phase A: global scales ----------
                with nc.named_scope("phaseA"):
                    nc.vector.memset(acc, 0.0)
                    nc.vector.memset(sums, 0.0)
                    for it in range(IT):
                        for m, w in enumerate((wg, wu, wd)):
                            rt = wpool.tile([P, H], F32, tag="wrt", name=f"a{m}_{it}")
                            nc.sync.dma_start(rt, w[it])
                            part = wpool.tile([P, 1], F32, tag="pa", name=f"pa{m}_{it}")
                            nc.vector.tensor_reduce(
                                part, rt, axis=AX.X, op=ALU.add,
                                apply_absolute_value=True,
                            )
                            nc.vector.tensor_tensor(
                                acc[:, m : m + 1], acc[:, m : m + 1], part, op=ALU.add
                            )
                    for m in range(3):
                        allb = wpool.tile([P, 1], F32, tag="allb", name=f"allb{m}")
                        nc.gpsimd.partition_all_reduce(
                            allb, acc[:, m : m + 1], P, bass_isa.ReduceOp.add
                        )
                        nc.vector.tensor_copy(sums[0:1, m : m + 1], allb[0:1, 0:1])
                    nc.sync.dma_start(cc_in[:], sums[:])
                    nc.gpsimd.collective_compute(
                        "AllReduce", ALU.add, ins=[cc_in[:]], outs=[cc_out[:]],
                        replica_groups=rg,
                    )
                    nc.scalar.dma_start(gsums[:], cc_out[:])
                    nc.vector.tensor_scalar(
                        den[0:1, 0:3], gsums[0:1, 0:3], rn, EPS, ALU.mult, ALU.add
                    )
                    nc.vector.reciprocal(rden[0:1, 0:3], den[0:1, 0:3])
                    nc.vector.tensor_scalar(
                        s3[0:1, 0:3], gsums[0:1, 0:3], rn, None, ALU.mult
                    )
                    nc.vector.tensor_tensor(cprod, s3[0:1, 0:1], s3[0:1, 1:2], op=ALU.mult)
                    nc.vector.tensor_tensor(cprod, cprod, s3[0:1, 2:3], op=ALU.mult)
                    nc.gpsimd.partition_broadcast(rdenb, rden)
                    nc.gpsimd.partition_broadcast(cb, cprod)

                # ---------- shared emitters ----------
                def emit_gateup(i, lg, lu, xbs, inters, nm):
                    """gate/up matmuls + inter=pg*pu for both blocks of a pair."""
                    for k in range(2):
                        pg = pspool.tile([P, TB], F32, tag="ps", name=f"pg{nm}_{i}_{k}")
                        for h in range(HT):
                            nc.tensor.matmul(
                                pg, lhsT=lg[:, h * P : (h + 1) * P],
                                rhs=xbs[k][:, h, :],
                                start=(h == 0), stop=(h == HT - 1),
                            )
                        pu = pspool.tile([P, TB], F32, tag="ps", name=f"pu{nm}_{i}_{k}")
                        for h in range(HT):
                            nc.tensor.matmul(
                                pu, lhsT=lu[:, h * P : (h + 1) * P],
                                rhs=xbs[k][:, h, :],
                                start=(h == 0), stop=(h == HT - 1),
                            )
                        usb = mpool.tile([P, TB], F32, tag="usb", bufs=2,
                                         name=f"usb{nm}_{i}_{k}")
                        nc.vector.tensor_copy(usb, pu)
                        nc.vector.tensor_tensor(
                            inters[k][:, i, :], pg, usb, op=ALU.mult
                        )

                def emit_down(bp, inters, nm):
                    """down matmuls + scaled bf16 output + RS for pair bp."""
                    for hb in range(HB):
                        qdc = mpool.tile([P, IT, TB], BF16, tag="qdc", bufs=2,
                                         name=f"qdc{nm}_{hb}")
                        nc.sync.dma_start(qdc, qd_d[:, :, hb * TB : (hb + 1) * TB])
                        for k in range(2):
                            b = 2 * bp + k
                            pos = [
                                pspool.tile([P, TB], F32, tag="ps",
                                            name=f"po{nm}_{hb}_{k}_{t}")
                                for t in range(TS)
                            ]
                            for i in range(IT):
                                for t in range(TS):
                                    nc.tensor.matmul(
                                        pos[t],
                                        lhsT=inters[k][:, i, t * P : (t + 1) * P],
                                        rhs=qdc[:, i, :],
                                        start=(i == 0), stop=(i == IT - 1),
                                    )
                            ob = mpool.tile([P, TS, TB], BF16, tag="ob", bufs=1,
                                            name=f"ob{nm}_{hb}_{k}")
                            for t in range(TS):
                                nc.vector.tensor_scalar(
                                    ob[:, t, :], pos[t], cb[:, 0:1], None, ALU.mult
                                )
                            nc.sync.dma_start(
                                outb[b][:, hb * TB : (hb + 1) * TB].rearrange(
                                    "(g p) f -> p g f", p=P
                                ),
                                ob,
                            )
                    for k in range(2):
                        b = 2 * bp + k
                        nc.gpsimd.collective_compute(
                            "ReduceScatter", ALU.add, ins=[outb[b][:]],
                            outs=[rsb[b][:]], replica_groups=rg,
                        )
                        nc.scalar.dma_start(y[b], rsb[b][:])

                # ---------- Phase B + pair 0 (B chased tile-by-tile) ----------
                with nc.named_scope("pair0"):
                    inter0 = [mpool.tile([P, IT, TB], BF16, tag=f"int{k}", bufs=1,
                                         name=f"int0_{k}") for k in range(2)]
                    for it in range(IT):
                        rts = []
                        for m, w in enumerate((wg, wu, wd)):
                            rt = wpool.tile([P, H], F32, tag="wrt", name=f"b{m}_{it}")
                            nc.sync.dma_start(rt, w[it])
                            rts.append(rt)
                        for m, rt in enumerate(rts):
                            nc.scalar.activation(
                                rt, rt, ACTF.Copy, bias=MAGIC,
                                scale=rdenb[:, m : m + 1],
                            )
                            nc.vector.tensor_scalar(
                                rt, rt, MAGIC, -1.0, ALU.subtract, ALU.max
                            )
                        qbg = qpool.tile([P, H], BF16, tag="qbg", name=f"qbg{it}")
                        nc.gpsimd.tensor_scalar(qbg, rts[0], 1.0, None, ALU.min)
                        qbu = qpool.tile([P, H], BF16, tag="qbu", name=f"qbu{it}")
                        nc.gpsimd.tensor_scalar(qbu, rts[1], 1.0, None, ALU.min)
                        qbd = qpool.tile([P, H], BF16, tag="qbd", bufs=1,
                                         name=f"qbd{it}")
                        nc.vector.tensor_scalar(qbd, rts[2], 1.0, None, ALU.min)
                        nc.sync.dma_start(qg_d[it], qbg)
                        nc.sync.dma_start(qu_d[it], qbu)
                        nc.sync.dma_start(qd_d[:, it, :], qbd)
                        # pair-0 consumes the quantized tiles straight from SBUF
                        emit_gateup(it, qbg, qbu, xb0, inter0, "p0")
                    emit_down(0, inter0, "p0")

                # ---------- pairs 1..NPAIR-1 ----------
                for bp in range(1, NPAIR):
                    with nc.named_scope(f"pair{bp}"):
                        xbs = [mpool.tile([P, HT, TB], BF16, tag=f"xb{k}", bufs=1,
                                          name=f"xb{bp}_{k}") for k in range(2)]
                        for k in range(2):
                            b = 2 * bp + k
                            nc.sync.dma_start(
                                xbs[k],
                                xTb[:, b * TB : (b + 1) * TB].rearrange(
                                    "(g p) f -> p g f", p=P
                                ),
                            )
                        inters = [mpool.tile([P, IT, TB], BF16, tag=f"int{k}", bufs=1,
                                             name=f"int{bp}_{k}") for k in range(2)]
                        for i in range(IT):
                            qgc = qpool.tile([P, H], BF16, tag="qbg", name=f"qgc{bp}_{i}")
                            nc.sync.dma_start(qgc, qg_d[i])
                            quc = qpool.tile([P, H], BF16, tag="qbu", name=f"quc{bp}_{i}")
                            nc.sync.dma_start(quc, qu_d[i])
                            emit_gateup(i, qgc, quc, xbs, inters, f"p{bp}")
                        emit_down(bp, inters, f"p{bp}")
    nc.compile()
    return nc


_NC_CACHE = {}


def _get_nc(T, H, I_real, n_cores):
    key = (T, H, I_real, n_cores)
    if key not in _NC_CACHE:
        _NC_CACHE[key] = build_bass(T, H, I_real, n_cores)
    return _NC_CACHE[key]


def shard_inputs(hidden_states, w_gate, w_up, w_down, n_cores=N_CORES):
    """Host layout: transpose / zero-pad / i-tile-major block / slice;
    activations cast to bf16 (bit-identical to an on-device cast)."""
    B, S, H = hidden_states.shape
    T = B * S
    I_real = w_gate.shape[0]
    i_s, IT = shard_sizes(I_real, n_cores)
    Ip = i_s * n_cores
    bf16 = mybir.dt.np(BF16)

    xTb = np.ascontiguousarray(
        hidden_states.reshape(T, H).T.astype(np.float32, copy=False)
    ).astype(bf16)

    def blk_gu(w):  # [I, H] -> per-core [IT, P, H] with [it,p,g*P+c]=w.T[g*P+p,it*P+c]
        wp = np.zeros((Ip, H), np.float32)
        wp[:I_real] = w
        out = []
        for c in range(n_cores):
            sh = wp[c * i_s : (c + 1) * i_s]
            out.append(
                np.ascontiguousarray(
                    sh.reshape(IT, P, H // P, P).transpose(0, 3, 2, 1).reshape(IT, P, H)
                )
            )
        return out

    wgs = blk_gu(w_gate)
    wus = blk_gu(w_up)
    wdp = np.zeros((Ip, H), np.float32)
    wdp[:I_real] = w_down.T
    wds = [
        np.ascontiguousarray(wdp[c * i_s : (c + 1) * i_s].reshape(IT, P, H))
        for c in range(n_cores)
    ]

    in_maps = []
    for c in range(n_cores):
        in_maps.append({"xTb": xTb, "wg": wgs[c], "wu": wus[c], "wd": wds[c]})
    return in_maps, (B, S, H, T)


def kernel(hidden_states, w_gate, w_up, w_down, _trace=False):
    global LAST_RESULTS
    n_cores = N_CORES
    in_maps, (B, S, H, T) = shard_inputs(hidden_states, w_gate, w_up, w_down, n_cores)
    I_real = w_gate.shape[0]
    nc = _get_nc(T, H, I_real, n_cores)
    res = run_bass_kernel_spmd(
        nc, in_maps, core_ids=list(range(n_cores)), trace=_trace
    )
    LAST_RESULTS = res

    NB = T // TB
    rq = TB // n_cores
    out = np.empty((T, H), np.float32)
    for c in range(n_cores):
        yc = res.results[c]["y"]  # [NB, rq, H] bf16
        yc = np.asarray(yc).astype(np.float32)
        for b in range(NB):
            out[b * TB + c * rq : b * TB + (c + 1) * rq] = yc[b]
    return out.reshape(B, S, H)


# revision 6
# speedup vs baseline: 1.3131x; 1.2388x over previous
"""Trainium2 Bass kernel for nn_LlamaMLP (BitLinear-style ternary-quantized MLP).

Reference computation (all f32):
    s_m   = mean(|w_m|)                            (global scalar per weight)
    q_m   = round(clip(w_m / (s_m + eps), -1, 1))  (ternary)
    gate  = x @ (q_g * s_g).T ; up = x @ (q_u * s_u).T
    out   = (gate * up) @ (q_d * s_d).T
        == (s_g*s_u*s_d) * ((x @ q_g.T) * (x @ q_u.T)) @ q_d.T

Strategy: tensor-parallel over the intermediate dim I (padded to a multiple of
128*n_cores).  Per core:

  Phase A   stream all three f32 weight shards once, reduce |w| partial sums
            (DVE), partition-reduce (GPSIMD), one 8-core AllReduce -> global
            scales.  Pair-0 x blocks prefetch concurrently on the scalar
            HWDGE queue.
  Phase B   re-stream the shards i-tile-major and quantize to ternary bf16:
            ACT (w*rden + MAGIC), DVE (sub MAGIC + clamp lo, in place), then
            GPSIMD (clamp hi + bf16 cast) for gate/up or DVE for down.
            Engine split keeps DVE/PE free of B back-pressure.
  Phase C   token-block-pair compute.  Pair 0 consumes the quantized gate/up
            tiles straight out of SBUF, chasing phase B tile-by-tile; later
            pairs re-read them from DRAM (written once by B).  Gate/up
            matmuls accumulate over H into PSUM; inter = pg*pu (DVE) in bf16;
            down matmuls accumulate over I; bf16 partial outputs are
            ReduceScatter'd per 512-token block (pipelined behind compute).

The host wrapper does layout only (transpose / zero-pad / block / concat plus
the f32->bf16 x cast, bit-identical to an on-device cast; weights stay f32 so
on-device quantization matches the reference).
"""

import sys

sys.path.insert(0, "/opt/trn_rl_repo")

import numpy as np
import concourse.mybir as mybir
import concourse.tile as tile
import concourse.bass_isa as bass_isa
from concourse import bacc
from concourse.bass_utils import run_bass_kernel_spmd

F32 = mybir.dt.float32
BF16 = mybir.dt.bfloat16
ALU = mybir.AluOpType
AX = mybir.AxisListType
ACTF = mybir.ActivationFunctionType

P = 128
TB = 512  # token-block width (matmul moving free dim)
MAGIC = 12582912.0  # 1.5*2^23; add+sub rounds an f32 to nearest-even integer
EPS = 1e-5

FULL_T, FULL_H, FULL_I = 8192, 4096, 11008
N_CORES = 8

LAST_RESULTS = None  # read by test.py


def shard_sizes(I_real, n_cores):
    i_s = -(-I_real // (P * n_cores)) * P  # per-core padded shard (mult of 128)
    return i_s, i_s // P


def build_bass(T=FULL_T, H=FULL_H, I_real=FULL_I, n_cores=N_CORES):
    assert T % (2 * TB) == 0 and H % P == 0 and H % TB == 0 and TB % n_cores == 0
    HT = H // P  # contraction tiles for gate/up
    HB = H // TB  # down-phase output column blocks
    NB = T // TB  # token blocks
    NPAIR = NB // 2
    TS = TB // P  # token sub-tiles per block (down-phase lhsT)
    i_s, IT = shard_sizes(I_real, n_cores)
    nreal = I_real * H
    rq = TB // n_cores  # ReduceScatter rows per core per block
    rn = 1.0 / float(nreal)

    nc = bacc.Bacc("TRN2", target_bir_lowering=False, debug=False, num_devices=n_cores)
    # i-tile-major blocked weights: w*[it][p, g*P+c] = w^T[g*P+p, it*P+c]
    xTb = nc.dram_tensor("xTb", [H, T], BF16, kind="ExternalInput")
    wg = nc.dram_tensor("wg", [IT, P, H], F32, kind="ExternalInput")
    wu = nc.dram_tensor("wu", [IT, P, H], F32, kind="ExternalInput")
    wd = nc.dram_tensor("wd", [IT, P, H], F32, kind="ExternalInput")
    y = nc.dram_tensor("y", [NB, rq, H], BF16, kind="ExternalOutput")
    rg = [list(range(n_cores))]

    with tile.TileContext(nc) as tc:
        with tc.tile_pool(name="dram", bufs=1, space="DRAM") as dram:
            qg_d = [dram.tile([P, H], BF16, name=f"qg{i}", tag=f"qg{i}") for i in range(IT)]
            qu_d = [dram.tile([P, H], BF16, name=f"qu{i}", tag=f"qu{i}") for i in range(IT)]
            qd_d = dram.tile([P, IT, H], BF16)
            outb = [
                dram.tile([TB, H], BF16, name=f"outb{b}", tag=f"outb{b}")
                for b in range(NB)
            ]
            rsb = [
                dram.tile([rq, H], BF16, name=f"rsb{b}", tag=f"rsb{b}")
                for b in range(NB)
            ]
            cc_in = dram.tile([1, 8], F32)
            cc_out = dram.tile([1, 8], F32, addr_space="Shared")

            with (
                tc.tile_pool(name="res", bufs=1) as rpool,
                tc.tile_pool(name="wp", bufs=3) as wpool,
                tc.tile_pool(name="qp", bufs=2) as qpool,
                tc.tile_pool(name="mp", bufs=1) as mpool,
                tc.tile_pool(name="ps", bufs=8, space="PSUM") as pspool,
            ):
                rdenb = rpool.tile([P, 4], F32)  # 1/(s_m+eps) bcast (cols g,u,d)
                cb = rpool.tile([P, 1], F32)  # s_g*s_u*s_d bcast
                acc = rpool.tile([P, 4], F32)
                sums = rpool.tile([1, 8], F32)
                gsums = rpool.tile([1, 8], F32)
                den = rpool.tile([1, 4], F32)
                rden = rpool.tile([1, 4], F32)
                s3 = rpool.tile([1, 4], F32)
                cprod = rpool.tile([1, 1], F32)

                # ---------- Phase A: global scales ----------
                with nc.named_scope("phaseA"):
                    nc.vector.memset(acc, 0.0)
                    nc.vector.memset(sums, 0.0)
                    for it in range(IT):
                        for m, w in enumerate((wg, wu, wd)):
                            rt = wpool.tile([P, H], F32, tag="wrt", name=f"a{m}_{it}")
                            nc.sync.dma_start(rt, w[it])
                            part = wpool.tile([P, 1], F32, tag="pa", name=f"pa{m}_{it}")
                            nc.vector.tensor_reduce(
                                part, rt, axis=AX.X, op=ALU.add,
                                apply_absolute_value=True,
                            )
                            nc.vector.tensor_tensor(
                                acc[:, m : m + 1], acc[:, m : m + 1], part, op=ALU.add
                            )
                    for m in range(3):
                        allb = wpool.tile([P, 1], F32, tag="allb", name=f"allb{m}")
                        nc.gpsimd.partition_all_reduce(
                            allb, acc[:, m : m + 1], P, bass_isa.ReduceOp.add
                        )
                        nc.vector.tensor_copy(sums[0:1, m : m + 1], allb[0:1, 0:1])
                    # pair-0 x blocks: issue right after A's weight stream
                    xb0 = [mpool.tile([P, HT, TB], BF16, tag=f"xb{k}", bufs=1,
                                      name=f"xb0_{k}") for k in range(2)]
                    for k in range(2):
                        nc.sync.dma_start(
                            xb0[k],
                            xTb[:, k * TB : (k + 1) * TB].rearrange(
                                "(g p) f -> p g f", p=P
                            ),
                        )
                    nc.sync.dma_start(cc_in[:], sums[:])
                    nc.gpsimd.collective_compute(
                        "AllReduce", ALU.add, ins=[cc_in[:]], outs=[cc_out[:]],
                        replica_groups=rg,
                    )
                    nc.scalar.dma_start(gsums[:], cc_out[:])
                    nc.vector.tensor_scalar(
                        den[0:1, 0:3], gsums[0:1, 0:3], rn, EPS, ALU.mult, ALU.add
                    )
                    nc.vector.reciprocal(rden[0:1, 0:3], den[0:1, 0:3])
                    nc.vector.tensor_scalar(
                        s3[0:1, 0:3], gsums[0:1, 0:3], rn, None, ALU.mult
                    )
                    nc.vector.tensor_tensor(cprod, s3[0:1, 0:1], s3[0:1, 1:2], op=ALU.mult)
                    nc.vector.tensor_tensor(cprod, cprod, s3[0:1, 2:3], op=ALU.mult)
                    nc.gpsimd.partition_broadcast(rdenb, rden)
                    nc.gpsimd.partition_broadcast(cb, cprod)

                # ---------- shared emitters ----------
                def emit_gateup(i, lg, lu, xbs, inters, nm):
                    """gate/up matmuls + inter=pg*pu for both blocks of a pair."""
                    for k in range(2):
                        pg = pspool.tile([P, TB], F32, tag="ps", name=f"pg{nm}_{i}_{k}")
                        for h in range(HT):
                            nc.tensor.matmul(
                                pg, lhsT=lg[:, h * P : (h + 1) * P],
                                rhs=xbs[k][:, h, :],
                                start=(h == 0), stop=(h == HT - 1),
                            )
                        pu = pspool.tile([P, TB], F32, tag="ps", name=f"pu{nm}_{i}_{k}")
                        for h in range(HT):
                            nc.tensor.matmul(
                                pu, lhsT=lu[:, h * P : (h + 1) * P],
                                rhs=xbs[k][:, h, :],
                                start=(h == 0), stop=(h == HT - 1),
                            )
                        usb = mpool.tile([P, TB], F32, tag="usb", bufs=2,
                                         name=f"usb{nm}_{i}_{k}")
                        nc.vector.tensor_copy(usb, pu)
                        nc.vector.tensor_tensor(
                            inters[k][:, i, :], pg, usb, op=ALU.mult
                        )

                def emit_down(bp, inters, nm):
                    """down matmuls + scaled bf16 output + RS for pair bp."""
                    for hb in range(HB):
                        qdc = mpool.tile([P, IT, TB], BF16, tag="qdc", bufs=2,
                                         name=f"qdc{nm}_{hb}")
                        nc.sync.dma_start(qdc, qd_d[:, :, hb * TB : (hb + 1) * TB])
                        for k in range(2):
                            b = 2 * bp + k
                            pos = [
                                pspool.tile([P, TB], F32, tag="ps",
                                            name=f"po{nm}_{hb}_{k}_{t}")
                                for t in range(TS)
                            ]
                            for i in range(IT):
                                for t in range(TS):
                                    nc.tensor.matmul(
                                        pos[t],
                                        lhsT=inters[k][:, i, t * P : (t + 1) * P],
                                        rhs=qdc[:, i, :],
                                        start=(i == 0), stop=(i == IT - 1),
                                    )
                            ob = mpool.tile([P, TS, TB], BF16, tag="ob", bufs=1,
                                            name=f"ob{nm}_{hb}_{k}")
                            for t in range(TS):
                                nc.vector.tensor_scalar(
                                    ob[:, t, :], pos[t], cb[:, 0:1], None, ALU.mult
                                )
                            nc.sync.dma_start(
                                outb[b][:, hb * TB : (hb + 1) * TB].rearrange(
                                    "(g p) f -> p g f", p=P
                                ),
                                ob,
                            )
                    for k in range(2):
                        b = 2 * bp + k
                        nc.gpsimd.collective_compute(
                            "ReduceScatter", ALU.add, ins=[outb[b][:]],
                            outs=[rsb[b][:]], replica_groups=rg,
                        )
                        nc.scalar.dma_start(y[b], rsb[b][:])

                # ---------- Phase B + pair 0 (B chased tile-by-tile) ----------
                with nc.named_scope("pair0"):
                    inter0 = [mpool.tile([P, IT, TB], BF16, tag=f"int{k}", bufs=1,
                                         name=f"int0_{k}") for k in range(2)]
                    for it in range(IT):
                        rts = []
                        for m, w in enumerate((wg, wu, wd)):
                            rt = wpool.tile([P, H], F32, tag="wrt", name=f"b{m}_{it}")
                            nc.sync.dma_start(rt, w[it])
                            rts.append(rt)
                        qbs = []
                        for m, rt in enumerate(rts):
                            nc.scalar.activation(
                                rt, rt, ACTF.Copy, bias=MAGIC,
                                scale=rdenb[:, m : m + 1],
                            )
                            nc.vector.tensor_scalar(
                                rt, rt, MAGIC, -1.0, ALU.subtract, ALU.max
                            )
                            qb = qpool.tile(
                                [P, H], BF16, tag=("qbg", "qbu", "qbd")[m],
                                bufs=(2, 2, 1)[m], name=f"qb{m}_{it}",
                            )
                            nc.vector.tensor_scalar(qb, rt, 1.0, None, ALU.min)
                            qbs.append(qb)
                        qbg, qbu, qbd = qbs
                        nc.sync.dma_start(qg_d[it], qbg)
                        nc.sync.dma_start(qu_d[it], qbu)
                        nc.sync.dma_start(qd_d[:, it, :], qbd)
                        # pair-0 consumes the quantized tiles straight from SBUF
                        emit_gateup(it, qbg, qbu, xb0, inter0, "p0")
                    emit_down(0, inter0, "p0")

                # ---------- pairs 1..NPAIR-1 ----------
                for bp in range(1, NPAIR):
                    with nc.named_scope(f"pair{bp}"):
                        xbs = [mpool.tile([P, HT, TB], BF16, tag=f"xb{k}", bufs=1,
                                          name=f"xb{bp}_{k}") for k in range(2)]
                        for k in range(2):
                            b = 2 * bp + k
                            nc.sync.dma_start(
                                xbs[k],
                                xTb[:, b * TB : (b + 1) * TB].rearrange(
                                    "(g p) f -> p g f", p=P
                                ),
                            )
                        inters = [mpool.tile([P, IT, TB], BF16, tag=f"int{k}", bufs=1,
                                             name=f"int{bp}_{k}") for k in range(2)]
                        for i in range(IT):
                            qgc = qpool.tile([P, H], BF16, tag="qbg", name=f"qgc{bp}_{i}")
                            nc.sync.dma_start(qgc, qg_d[i])
                            quc = qpool.tile([P, H], BF16, tag="qbu", name=f"quc{bp}_{i}")
                            nc.sync.dma_start(quc, qu_d[i])
                            emit_gateup(i, qgc, quc, xbs, inters, f"p{bp}")
                        emit_down(bp, inters, f"p{bp}")
    nc.compile()
    return nc


_NC_CACHE = {}


def _get_nc(T, H, I_real, n_cores):
    key = (T, H, I_real, n_cores)
    if key not in _NC_CACHE:
        _NC_CACHE[key] = build_bass(T, H, I_real, n_cores)
    return _NC_CACHE[key]


def shard_inputs(hidden_states, w_gate, w_up, w_down, n_cores=N_CORES):
    """Host layout: transpose / zero-pad / i-tile-major block / slice;
    activations cast to bf16 (bit-identical to an on-device cast)."""
    B, S, H = hidden_states.shape
    T = B * S
    I_real = w_gate.shape[0]
    i_s, IT = shard_sizes(I_real, n_cores)
    Ip = i_s * n_cores
    bf16 = mybir.dt.np(BF16)

    xTb = np.ascontiguousarray(
        hidden_states.reshape(T, H).T.astype(np.float32, copy=False)
    ).astype(bf16)

    def blk_gu(w):  # [I, H] -> per-core [IT, P, H] with [it,p,g*P+c]=w.T[g*P+p,it*P+c]
        wp = np.zeros((Ip, H), np.float32)
        wp[:I_real] = w
        out = []
        for c in range(n_cores):
            sh = wp[c * i_s : (c + 1) * i_s]
            out.append(
                np.ascontiguousarray(
                    sh.reshape(IT, P, H // P, P).transpose(0, 3, 2, 1).reshape(IT, P, H)
                )
            )
        return out

    wgs = blk_gu(w_gate)
    wus = blk_gu(w_up)
    wdp = np.zeros((Ip, H), np.float32)
    wdp[:I_real] = w_down.T
    wds = [
        np.ascontiguousarray(wdp[c * i_s : (c + 1) * i_s].reshape(IT, P, H))
        for c in range(n_cores)
    ]

    in_maps = []
    for c in range(n_cores):
        in_maps.append({"xTb": xTb, "wg": wgs[c], "wu": wus[c], "wd": wds[c]})
    return in_maps, (B, S, H, T)


def kernel(hidden_states, w_gate, w_up, w_down, _trace=False):
    global LAST_RESULTS
    n_cores = N_CORES
    in_maps, (B, S, H, T) = shard_inputs(hidden_states, w_gate, w_up, w_down, n_cores)
    I_real = w_gate.shape[0]
    nc = _get_nc(T, H, I_real, n_cores)
    res = run_bass_kernel_spmd(
        nc, in_maps, core_ids=list(range(n_cores)), trace=_trace
    )
    LAST_RESULTS = res

    NB = T // TB
    rq = TB // n_cores
    out = np.empty((T, H), np.float32)
    for c in range(n_cores):
        yc = res.results[c]["y"]  # [NB, rq, H] bf16
        yc = np.asarray(yc).astype(np.float32)
        for b in range(NB):
            out[b * TB + c * rq : b * TB + (c + 1) * rq] = yc[b]
    return out.reshape(B, S, H)
